# revision 14
# baseline (speedup 1.0000x reference)
"""Trainium2 Bass kernel for topk_masking (nn_CustomModule_8065948582484).

Reference semantics (per batch b):
  idx[b,f] = argmax(score[b,f,:196])                 (first index on ties)
  mask grows from a fixed prior region on a 14x14 grid; at frame f the
  argmax cell is added iff it is 4-adjacent to the current mask.
  out = [ones(B,1), masks frame-major] -> [B, 1+16*196] fp32.

Strategy (pure batch data-parallel across 8 cores, 2048 batches/core,
16 tiles of 128 batches on SBUF partitions):
  1. argmax via prefix-max scan with guard slots + popcount of
     (prefix_max < max) -- exact incl. first-index tie semantics.
  2. row index via popcount over row-end prefix-max slots; col = idx-14r.
  3. the sequential frame recurrence runs on a tiny 16-node adjacency
     graph (one fused tensor_tensor_reduce per frame on [128,16]).
  4. masks built by one-hot*added (tensor_scalar) + running max chain.
  5. ScalarE converts bf16 masks -> fp32 output; DMA in/out overlapped.
"""

import os
import sys

import numpy as np

for _p in ("/opt/trn_rl_repo",):
    if _p not in sys.path:
        sys.path.insert(0, _p)

from concourse import bacc, bass, mybir, tile  # noqa: E402
from concourse.bass_utils import run_bass_kernel_spmd  # noqa: E402

B, F, P = 16384, 16, 196
N = 14  # grid side
S = P + 1  # slots per frame in the scan layout (guard + 196)
NCORES = 8
BLOC = B // NCORES  # 2048
NT = BLOC // 128  # 16 tiles per core

ALU = mybir.AluOpType
AX = mybir.AxisListType
F32 = mybir.dt.float32
BF16 = mybir.dt.bfloat16
BIG = 1e30
# dev bisect knob: 1=scan, 2=+counts, 3=+rc, 4=+graph, 5=+recurrence,
# 6=+masks, 7=full (default)
STAGE = int(os.environ.get("K_STAGE", "7"))


def build_nc():
    nc = bacc.Bacc(trn_type="TRN2", target_bir_lowering=False)
    score_d = nc.declare_dram_parameter("score", [BLOC, F, P], F32, isOutput=False)
    out_d = nc.declare_dram_parameter("out", [BLOC, 1 + F * P], F32, isOutput=True)

    with tile.TileContext(nc) as tc:
        with (
            tc.tile_pool(name="consts", bufs=1) as cpool,
            tc.tile_pool(name="big", bufs=3) as bpool,
            tc.tile_pool(name="small", bufs=3) as spool,
        ):
            # ---- constants ----
            # iotap[q, p] = p  (0..195), exact in bf16
            iotap = cpool.tile([128, P], BF16, name="iotap")
            nc.gpsimd.iota(
                iotap[:],
                pattern=[[1, P]],
                base=0,
                channel_multiplier=0,
                allow_small_or_imprecise_dtypes=True,
            )
            # prior mask: rows 4..13, cols 2..11 set to 1
            prior = cpool.tile([128, P], BF16, name="prior")
            nc.vector.memset(prior[:], 0.0)
            priorv = prior.rearrange("q (r c) -> q r c", r=N)
            nc.vector.memset(priorv[:, 4:14, 2:12], 1.0)
            # scan reset pattern: +BIG everywhere, -BIG at each frame's guard slot
            d1 = cpool.tile([128, F * S], BF16, name="d1")
            nc.vector.memset(d1[:], BIG)
            d1v = d1.rearrange("q (f s) -> q f s", f=F)
            nc.vector.memset(d1v[:, :, 0:1], -BIG)

            def emit_diag(out_d, r0, bpool, diag_ap, n_el):
                out_t = bpool.tile(
                    [128, 1 + F * P], F32, tag="out", name="out_t"
                )
                nc.vector.memset(out_t[:], 0.0)
                nc.vector.tensor_copy(out_t[:, 1 : 1 + n_el], diag_ap)
                nc.sync.dma_start(out=out_d[r0 : r0 + 128, :], in_=out_t[:])

            for t in range(NT):
                r0 = t * 128
                # ---- load ----
                sc = bpool.tile([128, F * S], F32, tag="sc", name="sc")
                scv = sc.rearrange("q (f s) -> q f s", f=F)
                # guard slots must hold a finite value below all scores
                nc.gpsimd.memset(scv[:, :, 0:1], -BIG)
                nc.sync.dma_start(out=scv[:, :, 1:S], in_=score_d[r0 : r0 + 128])

                # ---- 1. prefix max with per-frame reset ----
                run = bpool.tile([128, F * S], F32, tag="run", name="run")
                nc.vector.tensor_tensor_scan(
                    run[:], sc[:], d1[:], 0.0, ALU.max, ALU.min
                )
                runv = run.rearrange("q (f s) -> q f s", f=F)
                if STAGE == 1:
                    emit_diag(out_d, r0, bpool, run[:, 0 : F * P], F * P)
                    continue

                # ---- 2. argmax index + row/col, all exact small ints in fp32 ----
                # idx = #positions whose prefix-max is still < max:
                # ScalarE computes sign(m - prefix_max) (+1 below max, 0 at/after
                # the first max) and its accumulator sums it along the frame.
                idxa = spool.tile([128, F], F32, tag="idxa", name="idxa")
                sjunk = spool.tile([128, P], BF16, tag="sjunk", name="sjunk")
                for f in range(F):
                    nc.scalar.activation(
                        sjunk[:],
                        runv[:, f, 1:S],
                        mybir.ActivationFunctionType.Sign,
                        bias=runv[:, f, P : P + 1],
                        scale=-1.0,
                        accum_out=idxa[:, f : f + 1],
                    )
                if STAGE == 2:
                    emit_diag(out_d, r0, bpool, idxa[:], F)
                    continue
                # r = #rows fully before the argmax row (prefix-max at row end < m)
                rowends = runv[:, :, N : S : N]  # [128, F, 14] slots 14,28,..,196
                m_b = runv[:, :, P : P + 1].broadcast_to([128, F, N])
                rlt = spool.tile([128, F, N], BF16, tag="rlt", name="rlt")
                nc.vector.tensor_tensor(rlt[:], rowends, m_b, ALU.is_lt)
                rr = spool.tile([128, F], F32, tag="rr", name="rr")
                nc.vector.tensor_reduce(rr[:], rlt[:], axis=AX.X, op=ALU.add)
                cc = spool.tile([128, F], F32, tag="cc", name="cc")
                nc.vector.scalar_tensor_tensor(
                    cc[:], rr[:], -float(N), idxa[:], ALU.mult, ALU.add
                )
                # v = 16*r + c  (pitch-16 cell id: adjacency <=> |dv| in {1,16})
                vv = spool.tile([128, F], F32, tag="vv", name="vv")
                nc.vector.scalar_tensor_tensor(
                    vv[:], rr[:], 16.0, cc[:], ALU.mult, ALU.add
                )

                if STAGE == 3:
                    emit_diag(out_d, r0, bpool, vv[:], F)
                    continue
                # ---- 3a. pairwise adjacency graph G[e,f] ----
                dv = spool.tile([128, F, F], F32, tag="dv", name="dv")
                nc.vector.tensor_tensor(
                    dv[:],
                    vv.unsqueeze(2).broadcast_to([128, F, F]),
                    vv.unsqueeze(1).broadcast_to([128, F, F]),
                    ALU.subtract,
                )
                adv = spool.tile([128, F, F], F32, tag="adv", name="adv")
                nc.vector.scalar_tensor_tensor(
                    adv[:], dv[:], -1.0, dv[:], ALU.mult, ALU.max
                )
                g1 = spool.tile([128, F, F], BF16, tag="g1", name="g1")
                nc.gpsimd.tensor_scalar(g1[:], adv[:], 1.0, None, ALU.is_equal)
                g16 = spool.tile([128, F, F], BF16, tag="g16", name="g16")
                nc.gpsimd.tensor_scalar(g16[:], adv[:], 16.0, None, ALU.is_equal)
                gg = spool.tile([128, F, F], BF16, tag="gg", name="gg")
                nc.vector.tensor_tensor(gg[:], g1[:], g16[:], ALU.add)

                # ---- 3b. adjacent-to-prior term A ----
                # A = (r>=3 & 2<=c<=11) | (r>=4 & 1<=c<=12)
                u3 = spool.tile([128, F], BF16, tag="u3", name="u3")
                nc.gpsimd.tensor_scalar(u3[:], rr[:], 3.0, None, ALU.is_ge)
                u4 = spool.tile([128, F], BF16, tag="u4", name="u4")
                nc.gpsimd.tensor_scalar(u4[:], rr[:], 4.0, None, ALU.is_ge)
                cm2 = spool.tile([128, F], F32, tag="cm2", name="cm2")
                nc.gpsimd.tensor_scalar(cm2[:], cc[:], 2.0, None, ALU.subtract)
                q1 = spool.tile([128, F], F32, tag="q1", name="q1")
                nc.vector.scalar_tensor_tensor(
                    q1[:], cc[:], -11.0, cm2[:], ALU.add, ALU.mult
                )
                b1 = spool.tile([128, F], BF16, tag="b1", name="b1")
                nc.gpsimd.tensor_scalar(b1[:], q1[:], 0.0, None, ALU.is_le)
                cm1 = spool.tile([128, F], F32, tag="cm1", name="cm1")
                nc.gpsimd.tensor_scalar(cm1[:], cc[:], 1.0, None, ALU.subtract)
                q2 = spool.tile([128, F], F32, tag="q2", name="q2")
                nc.vector.scalar_tensor_tensor(
                    q2[:], cc[:], -12.0, cm1[:], ALU.add, ALU.mult
                )
                b2 = spool.tile([128, F], BF16, tag="b2", name="b2")
                nc.gpsimd.tensor_scalar(b2[:], q2[:], 0.0, None, ALU.is_le)
                t1 = spool.tile([128, F], BF16, tag="t1", name="t1")
                nc.vector.tensor_tensor(t1[:], u3[:], b1[:], ALU.logical_and)
                t2 = spool.tile([128, F], BF16, tag="t2", name="t2")
                nc.vector.tensor_tensor(t2[:], u4[:], b2[:], ALU.logical_and)
                aa = spool.tile([128, F], F32, tag="aa", name="aa")
                nc.vector.tensor_tensor(aa[:], t1[:], t2[:], ALU.logical_or)

                if STAGE == 4:
                    emit_diag(out_d, r0, bpool, aa[:], F)
                    continue
                # ---- 3c. sequential added-recurrence over the 16-node graph ----
                added = spool.tile([128, F], F32, tag="added", name="added")
                nc.vector.memset(added[:], 0.0)
                t16 = spool.tile([128, F], BF16, tag="t16", name="t16")
                mx1 = spool.tile([128, 1], F32, tag="mx1", name="mx1")
                for f in range(F):
                    # added[f] = max(A[f], max_e added[e]*G[e,f])
                    nc.vector.tensor_tensor(t16[:], added[:], gg[:, :, f], ALU.mult)
                    nc.vector.tensor_reduce(mx1[:], t16[:], axis=AX.X, op=ALU.max)
                    nc.vector.tensor_tensor(
                        added[:, f : f + 1], mx1[:], aa[:, f : f + 1], ALU.max
                    )

                if STAGE == 5:
                    emit_diag(out_d, r0, bpool, added[:], F)
                    continue
                # ---- 4. build masks (bf16) ----
                # idxm[f] = added[f] ? idx[f] : -1   (-1 never matches iotap)
                idxp1 = spool.tile([128, F], F32, tag="idxp1", name="idxp1")
                nc.gpsimd.tensor_scalar(idxp1[:], idxa[:], 1.0, None, ALU.add)
                idxm = spool.tile([128, F], F32, tag="idxm", name="idxm")
                nc.vector.tensor_tensor(idxm[:], idxp1[:], added[:], ALU.mult)
                nc.vector.tensor_scalar(idxm[:], idxm[:], 1.0, None, ALU.subtract)
                masks = bpool.tile([128, F * P], BF16, tag="masks", name="masks")
                for f in range(F):
                    prev = prior[:] if f == 0 else masks[:, (f - 1) * P : f * P]
                    nc.vector.scalar_tensor_tensor(
                        masks[:, f * P : (f + 1) * P],
                        iotap[:],
                        idxm[:, f : f + 1],
                        prev,
                        ALU.is_equal,
                        ALU.max,
                    )

                if STAGE == 6:
                    emit_diag(out_d, r0, bpool, masks[:], F * P)
                    continue
                # ---- 5. convert to fp32 output and store ----
                out_t = bpool.tile([128, 1 + F * P], F32, tag="out", name="out_t")
                # ones column via ActE so the output DMA depends on one engine
                nc.scalar.activation(
                    out_t[:, 0:1],
                    d1[:, 0:1],
                    mybir.ActivationFunctionType.Copy,
                    bias=1.0,
                    scale=0.0,
                )
                nc.scalar.activation(
                    out_t[:, 1 : 1 + F * P],
                    masks[:],
                    mybir.ActivationFunctionType.Copy,
                )
                nc.sync.dma_start(out=out_d[r0 : r0 + 128, :], in_=out_t[:])

    nc.compile()
    return nc


_nc = None


def _get_nc():
    global _nc
    if _nc is None:
        _nc = build_nc()
    return _nc


def kernel(score, topn=196):
    score = np.ascontiguousarray(np.asarray(score, dtype=np.float32)).reshape(B, F, P)
    nc = _get_nc()
    in_maps = [
        {"score": score[i * BLOC : (i + 1) * BLOC]} for i in range(NCORES)
    ]
    res = run_bass_kernel_spmd(nc, in_maps, list(range(NCORES)))
    out = np.concatenate([res.results[i]["out"] for i in range(NCORES)], axis=0)
    return out
